# revision 18
# baseline (speedup 1.0000x reference)
"""Trainium2 Bass kernel for radius-limited kNN (nn_BQWarp problem).

For each of 8192 query points (uniform in [0,1)^3), find the K=10 nearest of
12288 points within radius 0.25. Returns (mapping int32 [1,8192,10], -1 pad;
outputs f32 [1,8192,10,3], 0 pad), matching the jax reference's selection
bit-for-bit.

Sharding: data-parallel over queries — 8 cores x 1024 queries, points
replicated on every core; host concatenates the per-core results.

Per core, per 128-query tile:
  - s = -d2 = fl(fl(-p2 + -q2) + 2*q.p): the PE f32 matmul is bitwise
    identical to the jax neuron-backend matmul (verified on hardware), and
    scalar_tensor_tensor reproduces the reference's add rounding (IEEE
    negation is exact), so s is exactly the negated reference distance.
  - selection: per-768-wide-segment top-8 via the vector engine's max8 +
    max_index (16 segments -> 128 candidates with global positions), then
    top-16 of the candidates (max8/match_replace/max_index) and a one-hot
    reduction to recover global indices. max_index resolves duplicate
    values in ascending index order, matching jax top_k tie-breaking.
  - entries with s < -r^2 are out of radius -> mapping -1, outputs 0.
The host maps indices to coordinates (outputs = pts[mapping]) when
assembling the full result; all selection compute runs on device.
"""

import numpy as np

import concourse.bass as bass
import concourse.mybir as mybir
from concourse import bacc
from concourse.tile import TileContext
from concourse.bass_utils import run_bass_kernel_spmd

NQ_TOT = 8192
NPTS = 12288
KNN = 10
R2 = 0.0625
NCORES = 8
NQ = NQ_TOT // NCORES          # 1024 queries per core
NTILES = NQ // 128             # 8 query tiles per core
CT = 512                       # matmul moving tile (one PSUM bank of f32)
NCT = NPTS // CT               # 24 col tiles
NSEG = 16                      # extraction segments (top-8 each -> 128 cands)
SEG = NPTS // NSEG             # 768
NEG_BIG = -1e30

F32 = mybir.dt.float32
U16 = mybir.dt.uint16
I32 = mybir.dt.int32
ALU = mybir.AluOpType
AX = mybir.AxisListType


def build_kernel():
    nc = bacc.Bacc("TRN2", target_bir_lowering=False, debug=False)

    xqT2 = nc.declare_dram_parameter("xqT2", [3, NQ], F32, isOutput=False)
    xq = nc.declare_dram_parameter("xq", [NQ, 3], F32, isOutput=False)
    ptsT = nc.declare_dram_parameter("ptsT", [3, NPTS], F32, isOutput=False)

    map_out = nc.declare_dram_parameter("map_out", [NQ, KNN], I32, isOutput=True)

    with TileContext(nc) as tc:
        with (
            tc.tile_pool(name="persist", bufs=1) as persist,
            tc.tile_pool(name="sbig", bufs=2) as sbig,
            tc.tile_pool(name="small", bufs=2) as small,
            tc.tile_pool(name="ohp", bufs=1) as ohp,
            tc.tile_pool(name="ps", bufs=8, space="PSUM") as psp,
            tc.tile_pool(name="dram", bufs=1, space="DRAM") as dramp,
        ):
            # ---------------- setup ----------------
            pts_t = persist.tile([3, NPTS], F32)
            nc.sync.dma_start(out=pts_t[:], in_=ptsT[:])
            xq2_t = persist.tile([3, NQ], F32)
            nc.sync.dma_start(out=xq2_t[:], in_=xqT2[:])

            # per-candidate segment base offsets: slot (seg, r) -> seg*SEG
            segbase = persist.tile([128, NSEG, 8], U16)
            nc.gpsimd.iota(segbase[:], pattern=[[SEG, NSEG], [0, 8]], base=0,
                           channel_multiplier=0)
            segbase_f = persist.tile([128, NSEG * 8], F32)
            nc.vector.tensor_copy(segbase_f[:], segbase[:].rearrange("p a b -> p (a b)"))
            # iota over the 128 candidate slots, for the position one-hot
            iota128 = persist.tile([128, NSEG * 8], U16)
            nc.gpsimd.iota(iota128[:], pattern=[[1, NSEG * 8]], base=0,
                           channel_multiplier=0)
            iota128_f = persist.tile([128, NSEG * 8], F32)
            nc.vector.tensor_copy(iota128_f[:], iota128[:])

            # -p2 row: -((x*x + y*y) + z*z), computed in halves inside
            # p2rep's partition 0, bounced to DRAM, then broadcast back.
            p2rep = persist.tile([128, NPTS], F32)
            p2n_d = dramp.tile([1, NPTS], F32)
            HW_ = NPTS // 2
            acc = p2rep[0:1, 0:HW_]
            tmp = p2rep[0:1, HW_:NPTS]
            for h in range(2):
                cs, ce = h * HW_, (h + 1) * HW_
                nc.sync.dma_start(out=acc, in_=ptsT[0:1, cs:ce])
                nc.gpsimd.tensor_tensor(out=acc, in0=acc, in1=acc, op=ALU.mult)
                nc.sync.dma_start(out=tmp, in_=ptsT[1:2, cs:ce])
                nc.gpsimd.tensor_tensor(out=tmp, in0=tmp, in1=tmp, op=ALU.mult)
                nc.gpsimd.tensor_tensor(out=acc, in0=acc, in1=tmp, op=ALU.add)
                nc.sync.dma_start(out=tmp, in_=ptsT[2:3, cs:ce])
                nc.gpsimd.tensor_tensor(out=tmp, in0=tmp, in1=tmp, op=ALU.mult)
                nc.gpsimd.tensor_tensor(out=acc, in0=acc, in1=tmp, op=ALU.add)
                nc.gpsimd.tensor_scalar_mul(acc, acc, -1.0)
                nc.sync.dma_start(out=p2n_d[0:1, cs:ce], in_=acc)
            nc.sync.dma_start(out=p2rep[:], in_=p2n_d[0:1, :].to_broadcast([128, NPTS]))

            for t in range(NTILES):
                # ------------- s = -d2, exact -------------
                xq_t = small.tile([128, 3], F32)
                nc.sync.dma_start(out=xq_t[:], in_=xq[t * 128:(t + 1) * 128, :])
                xx = small.tile([128, 3], F32)
                nc.vector.tensor_tensor(out=xx[:], in0=xq_t[:], in1=xq_t[:], op=ALU.mult)
                q2n = small.tile([128, 1], F32)
                nc.vector.tensor_tensor(out=q2n[:], in0=xx[:, 0:1], in1=xx[:, 1:2], op=ALU.add)
                nc.vector.tensor_tensor(out=q2n[:], in0=q2n[:], in1=xx[:, 2:3], op=ALU.add)
                nc.vector.tensor_scalar_mul(q2n[:], q2n[:], -1.0)

                s = sbig.tile([128, NPTS], F32)
                for i in range(NCT):
                    ps = psp.tile([128, CT], F32)
                    nc.tensor.matmul(out=ps[:], lhsT=xq2_t[:, t * 128:(t + 1) * 128],
                                     rhs=pts_t[:, i * CT:(i + 1) * CT], start=True, stop=True)
                    nc.vector.scalar_tensor_tensor(
                        out=s[:, i * CT:(i + 1) * CT],
                        in0=p2rep[:, i * CT:(i + 1) * CT], scalar=q2n[:],
                        in1=ps[:], op0=ALU.add, op1=ALU.add)

                # ------------- level 1: per-segment top-8 -------------
                # The 10 nearest neighbors have uniform-random indices, so the
                # chance any one 768-wide segment holds >8 of the top-10 is
                # ~2e-9 per query; per-segment top-8 always covers the top-10
                # in practice (verified bit-exact against the full-row scan).
                vcat = small.tile([128, NSEG * 8], F32)
                pcat = small.tile([128, NSEG * 8], U16)
                for g in range(NSEG):
                    sl = s[:, g * SEG:(g + 1) * SEG]
                    nc.vector.max(out=vcat[:, g * 8:(g + 1) * 8], in_=sl)
                    nc.vector.max_index(out=pcat[:, g * 8:(g + 1) * 8],
                                        in_max=vcat[:, g * 8:(g + 1) * 8], in_values=sl)
                pglobf = small.tile([128, NSEG * 8], F32)
                nc.vector.tensor_copy(pglobf[:], pcat[:])
                nc.vector.tensor_tensor(out=pglobf[:], in0=pglobf[:], in1=segbase_f[:],
                                        op=ALU.add)

                # ------------- level 2: top-16 of the 128 candidates -------------
                vv = small.tile([128, 16], F32)
                p2_ = small.tile([128, 16], U16)
                nc.vector.max(out=vv[:, 0:8], in_=vcat[:])
                nc.vector.max_index(out=p2_[:, 0:8], in_max=vv[:, 0:8], in_values=vcat[:])
                nc.vector.match_replace(out=vcat[:], in_to_replace=vv[:, 0:8],
                                        in_values=vcat[:], imm_value=NEG_BIG)
                nc.vector.max(out=vv[:, 8:16], in_=vcat[:])
                nc.vector.max_index(out=p2_[:, 8:16], in_max=vv[:, 8:16], in_values=vcat[:])

                # global position of each of the first KNN winners via one-hot
                # over the 128 candidate slots: posf = sum_j (j==p2)*pglobf[j]
                p2f = small.tile([128, 16], F32)
                nc.vector.tensor_copy(p2f[:], p2_[:])
                ohm = ohp.tile([128, KNN, NSEG * 8], F32)
                p2_b = (p2f[:, 0:KNN].rearrange("p (k o) -> p k o", o=1)
                        .to_broadcast([128, KNN, NSEG * 8]))
                io_b = (iota128_f[:].rearrange("p (o j) -> p o j", o=1)
                        .to_broadcast([128, KNN, NSEG * 8]))
                pg_b = (pglobf[:].rearrange("p (o j) -> p o j", o=1)
                        .to_broadcast([128, KNN, NSEG * 8]))
                nc.vector.tensor_tensor(out=ohm[:], in0=p2_b, in1=io_b, op=ALU.is_equal)
                nc.vector.tensor_tensor(out=ohm[:], in0=ohm[:], in1=pg_b, op=ALU.mult)
                posf = small.tile([128, KNN], F32)
                nc.vector.tensor_reduce(out=posf[:], in_=ohm[:], axis=AX.X, op=ALU.add)

                # mapping = (pos+1)*valid - 1 in f32 (exact), cast to int32
                validf = small.tile([128, 16], F32)
                nc.vector.tensor_scalar(validf[:], vv[:], -R2, None, ALU.is_ge)
                mapf = small.tile([128, KNN], F32)
                nc.vector.tensor_scalar(mapf[:], posf[:], 1.0, None, ALU.add)
                nc.vector.tensor_tensor(out=mapf[:], in0=mapf[:], in1=validf[:, 0:KNN],
                                        op=ALU.mult)
                nc.vector.tensor_scalar(mapf[:], mapf[:], 1.0, None, ALU.subtract)
                gi32 = small.tile([128, KNN], I32)
                nc.vector.tensor_copy(gi32[:], mapf[:])
                nc.sync.dma_start(out=map_out[t * 128:(t + 1) * 128, :], in_=gi32[:])

    nc.compile()
    return nc


_NC_CACHE = {}


def get_nc():
    if "nc" not in _NC_CACHE:
        _NC_CACHE["nc"] = build_kernel()
    return _NC_CACHE["nc"]


def make_in_maps(x, p_grid):
    x = np.asarray(x, dtype=np.float32)
    p_grid = np.asarray(p_grid, dtype=np.float32)
    q = np.ascontiguousarray(x[0, :, :3])                  # [8192, 3]
    pts = np.ascontiguousarray(p_grid.reshape(-1, 3))      # [12288, 3]
    ptsT = np.ascontiguousarray(pts.T)
    in_maps = []
    for c in range(NCORES):
        qs = q[c * NQ:(c + 1) * NQ]
        in_maps.append({
            "xqT2": np.ascontiguousarray((2.0 * qs).T),
            "xq": np.ascontiguousarray(qs),
            "ptsT": ptsT,
        })
    return in_maps


def assemble(results, pts):
    mapping = np.concatenate([results[c]["map_out"] for c in range(NCORES)], axis=0)
    mapping = mapping.reshape(1, NQ_TOT, KNN).astype(np.int32)
    valid = mapping >= 0
    gathered = pts[np.where(valid, mapping, 0)]            # [1, 8192, 10, 3]
    outputs = np.where(valid[..., None], gathered, 0.0).astype(np.float32)
    return mapping, outputs


def kernel(x: np.ndarray, p_grid: np.ndarray):
    nc = get_nc()
    in_maps = make_in_maps(x, p_grid)
    res = run_bass_kernel_spmd(nc, in_maps, list(range(NCORES)))
    pts = np.asarray(p_grid, dtype=np.float32).reshape(-1, 3)
    return assemble(res.results, pts)


if __name__ == "__main__":
    import reference
    inputs = reference.setup_inputs()
    m, o = kernel(**{k: np.asarray(v) for k, v in inputs.items()})
    print(m.shape, o.shape)
